# revision 35
# baseline (speedup 1.0000x reference)
"""Trainium2 Bass kernel for nn_AttentionalGNN (gnn_message_passing).

Algebraic collapse (exact, not approximate):

* In the reference, src[e] = x[row[row[e]]] and dst[e] = x[row[col[e]]] — every
  edge's attention inputs depend only on a node id < 200. The 4000x4000
  edge-attention therefore collapses exactly to a 200x200 attention with
  multiplicative column weights cb[w] = #{e : row[col[e]] = w} (applied as an
  additive ln(cb) bias on the logits), and the scatter-add collapses to a
  200x200 count matrix M[n,u] = #{e : col[e]=n, row[row[e]]=u}.

* The per-node "lin" layer is linear, so it folds into the q/k/v weights on
  the host; the 1/sqrt(C) scale folds into q.

* q/k fold further into a Gram matrix G_h = (scale*qlw_h)^T klw_h so scores
  are x^T G x directly; the per-query-constant logit terms cancel in softmax
  and are dropped; the per-key term folds into the exp bias.

* The output projection folds into v: VO_h = (ow_h vlw_h)^T, so the
  attention-output matmul directly produces o-projected messages; a ones
  column appended to vo makes the same matmul emit the softmax normalizer Z.

Everything runs feature-major / node-major so all softmax steps are
per-partition ops; the only broadcasts needed are host-precomputed.

The program is replicated SPMD on 8 cores (post-collapse the problem is tiny
and latency-dominated; collectives would only add overhead).
"""

import numpy as np

N = 200          # nodes
C = 128          # channels
H = 4            # heads
DH = 128         # head dim
QKV = 512        # H * DH
_CHUNKS = ((0, 128), (128, 72))   # (start, size) chunks of the node axis
N_CORES = 8
PACK1 = N + 512 + 4 + N           # xT | G1 | RV1 | lnc-row
PACK2 = QKV + QKV + C             # VO1 | boB1 | obB1
PACK3 = 2 * N + 512 + 4 + QKV + QKV + C   # MT0 | MT1 | G2 | RV2 | VO2 | boB2 | obB2

_CACHE = {}


def _build_program():
    import concourse.mybir as mybir
    import concourse.tile as tile
    from concourse import bacc

    dt = mybir.dt.float32
    AF = mybir.ActivationFunctionType
    AX = mybir.AxisListType
    OP = mybir.AluOpType

    nc = bacc.Bacc("TRN2", target_bir_lowering=False)

    din = {}

    def dram_in(name, shape):
        din[name] = nc.dram_tensor(name, shape, dt, kind="ExternalInput")

    # All inputs arrive packed into three [128, *] tensors so each is a single
    # contiguous DMA (first-byte latency paid ~once per queue, not ~20 times):
    #   pack1: xT | G1 (4 head blocks) | RV1 | lnc (row 0 only)
    #   pack2: VO1 | boB1 | obB1
    #   pack3: MT0 | MT1 | G2 | RV2 | VO2 | boB2 | obB2
    dram_in("pack1", [C, PACK1])
    dram_in("pack2", [C, PACK2])
    dram_in("pack3", [C, PACK3])
    y_d = nc.dram_tensor("yT", [C, N], dt, kind="ExternalOutput")

    with tile.TileContext(nc) as tc:
        with (
            tc.tile_pool(name="singles", bufs=1) as singles,
            tc.tile_pool(name="work", bufs=2) as work,
            tc.tile_pool(name="psum", bufs=8, space="PSUM") as psum,
        ):
            # constants + engine warm-up first: they have no DMA deps, so the
            # PE ramps and the ACT tables load during the initial weight DMAs
            junk = singles.tile([128, 288], dt, tag="w_junk")
            nc.vector.memset(junk[:], 1.0)
            ones_row = singles.tile([1, 512], dt, tag="w_ones_row")
            nc.gpsimd.memset(ones_row[:], 1.0)
            for wrows in (256, 256, 288):
                ps = psum.tile([128, 512], dt, tag="ps")
                nc.tensor.matmul(ps[:, :wrows], junk[:, :128],
                                 junk[:, :wrows], start=True, stop=True)
            warm = singles.tile([1, 1], dt, tag="w_warm")
            nc.scalar.activation(out=warm[:], in_=junk[:1, :1], func=AF.Ln)
            nc.scalar.activation(out=warm[:], in_=junk[:1, :1], func=AF.Exp)

            # packed loads: layer-1 tensors on the sync (HWDGE) queue,
            # layer-2 + aggregation tensors in parallel on the gpsimd queue
            p1 = singles.tile([C, PACK1], dt, tag="w_pack1", name="p1")
            # score-path tensors (xT|G1) land first, split across both DMA
            # queues by partition halves; RV1|lnc and pack2 follow on sync
            nc.sync.dma_start(p1[:96, :N + 512], din["pack1"][:96, :N + 512])
            nc.gpsimd.dma_start(p1[96:, :N + 512],
                                din["pack1"][96:, :N + 512])
            nc.sync.dma_start(p1[:, N + 512:], din["pack1"][:, N + 512:])
            p2 = singles.tile([C, PACK2], dt, tag="w_pack2", name="p2")
            nc.sync.dma_start(p2[:], din["pack2"][:])
            p3 = singles.tile([C, PACK3], dt, tag="w_pack3", name="p3")
            nc.gpsimd.dma_start(p3[:], din["pack3"][:])

            xT = p1[:, 0:N]
            lnc_row = p1[0:1, N + 516:N + 516 + N]
            W = {
                "G1": p1[:, N:N + 512],
                "RV1": p1[:, N + 512:N + 516],
                "VO1": p2[:, 0:512],
                "boB1": p2[:, 512:1024],
                "obB1": p2[:, 1024:1152],
                "MT0": p3[:, 0:N],
                "MT1": p3[:, N:2 * N],
                "G2": p3[:, 2 * N:2 * N + 512],
                "RV2": p3[:, 2 * N + 512:2 * N + 516],
                "VO2": p3[:, 2 * N + 516:2 * N + 1028],
                "boB2": p3[:, 2 * N + 1028:2 * N + 1540],
                "obB2": p3[:, 2 * N + 1540:2 * N + 1668],
            }
            MT_sb = [W["MT0"], W["MT1"]]

            def gnn_layer(L, x_in):
                """x_in: SBUF [C, N] feature-major. Returns PSUM tile whose
                [:, :N] holds the aggregated feature-major layer output."""
                # th_h = G_h^T x  (scores are th^T x)
                th = work.tile([C, H * N], dt, tag="th")
                for hp in range(2):
                    ps = psum.tile([128, 512], dt, tag="ps")
                    for hh in range(2):
                        h = hp * 2 + hh
                        nc.tensor.matmul(ps[:, hh * N:(hh + 1) * N],
                                         W[f"G{L}"][:, h * C:(h + 1) * C], x_in,
                                         start=True, stop=True)
                    nc.vector.tensor_copy(
                        out=th[:, hp * 2 * N:(hp + 1) * 2 * N],
                        in_=ps[:, :2 * N])

                # per-key exp bias: ln(cb) + rv_h . x_w  (lnc added via a K=1
                # rank-1 matmul: lhsT carries the per-partition values)
                bias_sb = []
                for ci, (w0, wc) in enumerate(_CHUNKS):
                    ps = psum.tile([128, 512], dt, tag="ps")
                    nc.tensor.matmul(ps[:wc, :H], x_in[:, w0:w0 + wc],
                                     W[f"RV{L}"][:], start=True, stop=False)
                    nc.tensor.matmul(ps[:wc, :H], lnc_row[:, w0:w0 + wc],
                                     ones_row[:1, :H], start=False, stop=True)
                    b = work.tile([128, H], dt, tag=f"bias{ci}")
                    nc.vector.tensor_copy(out=b[:wc, :], in_=ps[:wc, :H])
                    bias_sb.append(b)

                # scores + weighted exp, keys on partitions
                PT = []
                for ci, (w0, wc) in enumerate(_CHUNKS):
                    pt = work.tile([128, H * N], dt, tag=f"PT{ci}")
                    for hp in range(2):
                        ps = psum.tile([128, 512], dt, tag="ps")
                        nc.tensor.matmul(
                            ps[:wc, :2 * N],
                            x_in[:, w0:w0 + wc],
                            th[:, hp * 2 * N:(hp + 1) * 2 * N],
                            start=True, stop=True)
                        for hh in range(2):
                            h = hp * 2 + hh
                            nc.scalar.activation(
                                out=pt[:wc, h * N:(h + 1) * N],
                                in_=ps[:wc, hh * N:(hh + 1) * N],
                                func=AF.Exp,
                                bias=bias_sb[ci][:wc, h:h + 1], scale=1.0)
                    PT.append(pt)

                # vo = x^T VO + bo, node-major, with ones column appended per
                # head (so the attention matmul also emits Z in column DH)
                vo_sb = []
                for ci, (w0, wc) in enumerate(_CHUNKS):
                    ps = psum.tile([128, 512], dt, tag="ps")
                    nc.tensor.matmul(ps[:wc, :], x_in[:, w0:w0 + wc],
                                     W[f"VO{L}"][:], start=True, stop=True)
                    vt = work.tile([128, H, DH + 1], dt, tag=f"vo{ci}")
                    nc.vector.memset(vt[:wc, :, DH:], 1.0)
                    nc.vector.tensor_add(
                        out=vt[:wc, :, :DH],
                        in0=ps[:wc, :].rearrange("p (h c) -> p h c", h=H),
                        in1=W[f"boB{L}"][:wc, :].rearrange(
                            "p (h c) -> p h c", h=H))
                    vo_sb.append(vt)

                # attention x vo -> o-projected messages (node-major), with Z
                # in the extra column; normalize per head and sum heads
                msgs = []
                for ui, (u0, uc) in enumerate(_CHUNKS):
                    m = work.tile([128, C], dt, tag=f"msg{ui}")
                    rz = work.tile([128, H], dt, tag=f"rz{ui}")
                    for h in range(H):
                        ps = psum.tile([128, 512], dt, tag="ps")
                        for ci, (w0, wc) in enumerate(_CHUNKS):
                            nc.tensor.matmul(
                                ps[:uc, :DH + 1],
                                PT[ci][:wc, h * N + u0:h * N + u0 + uc],
                                vo_sb[ci][:wc, h, :],
                                start=(ci == 0), stop=(ci == 1))
                        nc.vector.reciprocal(out=rz[:uc, h:h + 1],
                                             in_=ps[:uc, DH:DH + 1])
                        nc.vector.scalar_tensor_tensor(
                            out=m[:uc, :],
                            in0=ps[:uc, :DH],
                            scalar=rz[:uc, h:h + 1],
                            in1=(W[f"obB{L}"][:uc, :] if h == 0 else m[:uc, :]),
                            op0=OP.mult, op1=OP.add)
                    msgs.append(m)

                # scatter-add collapse: outT = msg^T @ MT
                ps = psum.tile([128, 512], dt, tag="ps")
                for ui, (u0, uc) in enumerate(_CHUNKS):
                    nc.tensor.matmul(ps[:, :N], msgs[ui][:uc, :],
                                     MT_sb[ui][:uc, :],
                                     start=(ui == 0), stop=(ui == 1))
                return ps

            ps1 = gnn_layer(1, xT)
            hT = work.tile([C, N], dt, tag="hT")
            nc.vector.tensor_scalar_max(out=hT[:], in0=ps1[:, :N], scalar1=0.0)
            ps2 = gnn_layer(2, hT)

            # log_softmax over the node axis (free dim). No max-subtraction:
            # the pre-softmax outputs are bounded (|x| < ~20 for this model),
            # so exp stays well inside fp32 range and the result is identical
            # up to rounding.
            esum = work.tile([128, 1], dt, tag="esum")
            etmp = work.tile([128, N], dt, tag="etmp")
            nc.scalar.activation(out=etmp[:], in_=ps2[:, :N], func=AF.Exp,
                                 scale=1.0, accum_out=esum[:])
            lse = work.tile([128, 1], dt, tag="lse")
            nc.scalar.activation(out=lse[:], in_=esum[:], func=AF.Ln)
            out_sb = work.tile([128, N], dt, tag="out_sb")
            nc.vector.tensor_scalar_sub(out=out_sb[:], in0=ps2[:, :N],
                                        scalar1=lse[:])
            nc.sync.dma_start(y_d[:], out_sb[:])

    nc.compile()
    return nc


def _prep_inputs(x, edge_index, params):
    """Host-side preprocessing: index collapse + weight folding (float64)."""
    f32 = np.float32
    row = np.asarray(edge_index[0]).astype(np.int64)
    col = np.asarray(edge_index[1]).astype(np.int64)
    a = row[row]
    b = row[col]
    cb = np.bincount(b, minlength=N).astype(np.float64)
    lnc = np.where(cb > 0, np.log(np.maximum(cb, 1e-300)), -1e30)
    M = np.zeros((N, N), np.float64)
    np.add.at(M, (col, a), 1.0)

    folded = {}
    scale = np.float64(1.0) / np.sqrt(np.float64(C))
    for L in (1, 2):
        p = {k: np.asarray(params[f"l{L}_{k}"]).astype(np.float64)
             for k in ("lin_w", "lin_b", "q_w", "q_b", "k_w", "k_b",
                       "v_w", "v_b", "o_w", "o_b")}
        sqlw = (p["q_w"] @ p["lin_w"]) * scale           # [512, 128]
        sqlb = (p["q_w"] @ p["lin_b"] + p["q_b"]) * scale
        klw = p["k_w"] @ p["lin_w"]
        klb = p["k_w"] @ p["lin_b"] + p["k_b"]
        vlw = p["v_w"] @ p["lin_w"]
        vlb = p["v_w"] @ p["lin_b"] + p["v_b"]
        G = np.empty((C, H, C))
        RV = np.empty((C, H))
        VO = np.empty((C, QKV))
        bo = np.empty(QKV)
        for h in range(H):
            sl = slice(h * DH, (h + 1) * DH)
            G[:, h, :] = sqlw[sl].T @ klw[sl]
            RV[:, h] = klw[sl].T @ sqlb[sl]
            ow_h = p["o_w"][:, sl]                       # [C, DH]
            VO[:, sl] = vlw[sl].T @ ow_h.T
            bo[sl] = ow_h @ vlb[sl]
        folded[f"G{L}"] = G.reshape(C, H * C)   # head blocks along columns
        folded[f"RV{L}"] = RV
        folded[f"VO{L}"] = VO
        folded[f"boB{L}"] = np.tile(bo, (C, 1))
        folded[f"obB{L}"] = np.tile(p["o_b"], (C, 1))

    xT = np.asarray(x, np.float32)[0].T.astype(np.float64)
    lnc_block = np.zeros((C, N))
    lnc_block[0, :] = lnc
    pack1 = np.concatenate(
        [xT, folded["G1"], folded["RV1"], lnc_block], axis=1)
    pack2 = np.concatenate(
        [folded["VO1"], folded["boB1"], folded["obB1"]], axis=1)
    MT0 = M.T[0:128, :]
    MT1 = np.zeros((C, N))
    MT1[0:72, :] = M.T[128:200, :]
    pack3 = np.concatenate(
        [MT0, MT1, folded["G2"], folded["RV2"], folded["VO2"],
         folded["boB2"], folded["obB2"]], axis=1)
    assert pack1.shape == (C, PACK1) and pack2.shape == (C, PACK2) \
        and pack3.shape == (C, PACK3)
    return {
        "pack1": np.ascontiguousarray(pack1.astype(f32)),
        "pack2": np.ascontiguousarray(pack2.astype(f32)),
        "pack3": np.ascontiguousarray(pack3.astype(f32)),
    }


def run_on_device(in_map, trace=False, **kwargs):
    from concourse.bass_utils import run_bass_kernel_spmd

    if "nc" not in _CACHE:
        _CACHE["nc"] = _build_program()
    nc = _CACHE["nc"]
    res = run_bass_kernel_spmd(nc, [in_map] * N_CORES,
                               core_ids=list(range(N_CORES)),
                               trace=trace, **kwargs)
    return res


def kernel(x, edge_index, **params):
    in_map = _prep_inputs(x, edge_index, params)
    res = run_on_device(in_map)
    yT = res.results[0]["yT"]
    return np.ascontiguousarray(yT.T)[None].astype(np.float32)
